# revision 12
# baseline (speedup 1.0000x reference)
"""BitGRUCell kernel for 8 Trainium2 NeuronCores.

Strategy: data-parallel over the batch (B=4096 -> 512 rows/core), binary gate
weights replicated. Everything on device is kept feature-major ([features,
batch] = [partition, free]) so no transposes are needed anywhere:

  gate_acc[h, b] = sum_k signW.T[k, h] * actT[k, b]     (PE, bf16, fp32 acc)
  gate[h, b]     = act_fn(scale * gate_acc + bias[h])   (ScalarE, from PSUM)
  h_new          = hidden + z * (n - hidden)            (VectorE, fp32)

The per-tensor abs-mean scale of the binarized weights is applied via the
activation instruction's scale operand, so the device matmuls run on exact
{-1, 0, +1} bf16 weights.
"""

import numpy as np
import ml_dtypes

import concourse.bass as bass
import concourse.mybir as mybir
import concourse.tile as tile
from concourse import bacc
from concourse.bass import ts
from concourse.bass_utils import run_bass_kernel_spmd

B, I, H = 4096, 2048, 2048
NCORES = 8
BL = B // NCORES          # 512 batch rows per core
P = 128
KI = I // P               # 16 k-tiles in the x part
KH = H // P               # 16 k-tiles in the hidden part
KT = KI + KH              # 32 k-tiles per gate matmul
MT = H // P               # 16 output h-tiles
BF16 = mybir.dt.bfloat16
F32 = mybir.dt.float32
NPBF16 = ml_dtypes.bfloat16

_PROGRAM_CACHE = {}


def _build_program():
    nc = bacc.Bacc("TRN2", target_bir_lowering=False, debug=False,
                   num_devices=NCORES)

    xb_d = nc.dram_tensor("xb", [P, KI, BL], BF16, kind="ExternalInput")
    hb_d = nc.dram_tensor("hb", [P, KH, BL], BF16, kind="ExternalInput")
    hf_d = nc.dram_tensor("hf", [P, KH, BL], F32, kind="ExternalInput")
    wr_d = nc.dram_tensor("wr", [MT, P, KT * P], BF16, kind="ExternalInput")
    wz_d = nc.dram_tensor("wz", [MT, P, KT * P], BF16, kind="ExternalInput")
    wn_d = nc.dram_tensor("wn", [MT, P, KT * P], BF16, kind="ExternalInput")
    bias_d = nc.dram_tensor("bias", [P, 3 * MT], F32, kind="ExternalInput")
    scale_d = nc.dram_tensor("scale", [P, 3], F32, kind="ExternalInput")
    out_d = nc.dram_tensor("out", [P, MT, BL], F32, kind="ExternalOutput")

    SIG = mybir.ActivationFunctionType.Sigmoid
    TANH = mybir.ActivationFunctionType.Tanh

    with tile.TileContext(nc) as tc:
        with (
            tc.tile_pool(name="const", bufs=1) as cpool,
            tc.tile_pool(name="persist", bufs=1) as ppool,
            tc.tile_pool(name="wstream", bufs=5) as wpool,
            tc.tile_pool(name="acts", bufs=3) as apool,
            tc.tile_pool(name="outs", bufs=3) as opool,
            tc.tile_pool(name="psum", bufs=2, space="PSUM") as pspool,
        ):
            bias_t = cpool.tile([P, 3 * MT], F32)
            scale_t = cpool.tile([P, 3], F32)

            xb_t = ppool.tile([P, KI, BL], BF16)
            hb_t = ppool.tile([P, KH, BL], BF16)
            hf_t = ppool.tile([P, KH, BL], F32)
            z_t = ppool.tile([P, MT, BL], F32)
            rh_t = ppool.tile([P, KH, BL], BF16)

            # DMA queue order is FIFO: interleave the first weight tile's
            # chunks with the xb chunks in consumption order so the PE can
            # start as soon as the first ~0.8 MB lands, instead of waiting
            # for the whole 10 MB input set. The loop body consumes wr0
            # against xb first, then hb, then wz0 — match that order here.
            # hf (phase-B-only) trickles in during phase A.
            # PE warm-up: ~80 tiny matmuls on scratch SBUF with no DMA deps.
            # They run during the initial DMA wait and trip the HAM activity
            # monitor, so the real matmuls start at 2.4 GHz instead of
            # spending their first ~8 us at the cold 1.2 GHz half-rate.
            warm_t = cpool.tile([P, P], BF16)
            nc.gpsimd.memset(warm_t[:], 0)
            ps_w = pspool.tile([P, 64], F32, tag="pw", name="ps_w", bufs=1)
            for _ in range(200):
                nc.tensor.matmul(ps_w[:], warm_t[:], warm_t[:, :64],
                                 start=True, stop=True)

            CH = 4            # k-tiles per activation-load chunk
            WCH = 8           # k-tiles per first-weight-load chunk
            wr_first = wpool.tile([P, KT * P], BF16, tag="w", name="wr_first")
            for c in range(KI // CH):
                nc.sync.dma_start(wr_first[:, ts(c, WCH * P)],
                                  wr_d[0][:, ts(c, WCH * P)])
                nc.sync.dma_start(xb_t[:, ts(c, CH), :], xb_d[:, ts(c, CH), :])
            nc.sync.dma_start(bias_t[:], bias_d[:])
            nc.sync.dma_start(scale_t[:], scale_d[:])
            wz_first = wpool.tile([P, KT * P], BF16, tag="w", name="wz_first")
            for c in range(KH // CH):
                nc.sync.dma_start(hb_t[:, ts(c, CH), :], hb_d[:, ts(c, CH), :])
                nc.sync.dma_start(wz_first[:, ts(c, WCH * P)],
                                  wz_d[0][:, ts(c, WCH * P)])

            def rhs_for(k):
                return xb_t[:, k, :] if k < KI else hb_t[:, k - KI, :]

            # Phase A: r and z gates; rh = sigmoid(r_acc)*hidden kept in bf16.
            for m in range(MT):
                if m == 0:
                    wr_m = wr_first
                else:
                    wr_m = wpool.tile([P, KT * P], BF16, tag="w", name="wr_m")
                    nc.sync.dma_start(wr_m[:], wr_d[m])
                ps_r = pspool.tile([P, BL], F32, tag="pr", name="ps_r")
                for k in range(KT):
                    nc.tensor.matmul(ps_r[:], wr_m[:, ts(k, P)], rhs_for(k),
                                     start=(k == 0), stop=(k == KT - 1))
                r_m = apool.tile([P, BL], BF16, tag="r", name="r_m")
                nc.scalar.activation(r_m[:], ps_r[:], SIG,
                                     bias=bias_t[:, m:m + 1],
                                     scale=scale_t[:, 0:1])
                nc.vector.tensor_mul(rh_t[:, m, :], r_m[:], hb_t[:, m, :])

                # Trickle in the fp32 hidden copy (needed only in phase B).
                nc.sync.dma_start(hf_t[:, m, :], hf_d[:, m, :])

                if m == 0:
                    wz_m = wz_first
                else:
                    wz_m = wpool.tile([P, KT * P], BF16, tag="w", name="wz_m")
                    nc.sync.dma_start(wz_m[:], wz_d[m])
                ps_z = pspool.tile([P, BL], F32, tag="pz", name="ps_z")
                for k in range(KT):
                    nc.tensor.matmul(ps_z[:], wz_m[:, ts(k, P)], rhs_for(k),
                                     start=(k == 0), stop=(k == KT - 1))
                nc.scalar.activation(z_t[:, m, :], ps_z[:], SIG,
                                     bias=bias_t[:, MT + m:MT + m + 1],
                                     scale=scale_t[:, 1:2])

            # Phase B: n gate over [x, r*hidden]; h_new = h + z*(n - h).
            # The last m-tile is processed in two batch halves so its
            # activation/elementwise/store tail is half as long.
            for m in range(MT):
                wn_m = wpool.tile([P, KT * P], BF16, tag="w", name="wn_m")
                nc.sync.dma_start(wn_m[:], wn_d[m])
                halves = ((0, BL),) if m < MT - 1 else (
                    (0, BL // 2), (BL // 2, 3 * BL // 4), (3 * BL // 4, BL))
                for lo, hi in halves:
                    ps_n = pspool.tile([P, BL], F32, tag="pn", name="ps_n")
                    for k in range(KI):
                        nc.tensor.matmul(ps_n[:, :hi - lo],
                                         wn_m[:, ts(k, P)],
                                         xb_t[:, k, lo:hi],
                                         start=(k == 0), stop=False)
                    for k in range(KH):
                        nc.tensor.matmul(ps_n[:, :hi - lo],
                                         wn_m[:, ts(KI + k, P)],
                                         rh_t[:, k, lo:hi],
                                         start=False, stop=(k == KH - 1))
                    n_m = apool.tile([P, BL], F32, tag="n", name="n_m")
                    nc.scalar.activation(n_m[:, :hi - lo], ps_n[:, :hi - lo],
                                         TANH,
                                         bias=bias_t[:, 2 * MT + m:2 * MT + m + 1],
                                         scale=scale_t[:, 2:3])
                    d_m = apool.tile([P, BL], F32, tag="d", name="d_m")
                    nc.vector.tensor_sub(d_m[:, :hi - lo], n_m[:, :hi - lo],
                                         hf_t[:, m, lo:hi])
                    nc.vector.tensor_mul(d_m[:, :hi - lo], z_t[:, m, lo:hi],
                                         d_m[:, :hi - lo])
                    o_m = opool.tile([P, BL], F32, tag="o", name="o_m")
                    nc.vector.tensor_add(o_m[:, :hi - lo], hf_t[:, m, lo:hi],
                                         d_m[:, :hi - lo])
                    nc.sync.dma_start(out_d[:, m, lo:hi], o_m[:, :hi - lo])

    nc.finalize()
    return nc


def _get_program():
    if "nc" not in _PROGRAM_CACHE:
        _PROGRAM_CACHE["nc"] = _build_program()
    return _PROGRAM_CACHE["nc"]


def _prep_weight(w):
    # [H, I+H] fp32 -> sign -> bf16, tiled to [MT, P, KT*P] so that
    # wtile[m][p, k*P + f] = sign(w)[m*P + f, k*P + p]; each [P, KT*P] slice
    # is one contiguous 1 MB DMA whose partition dim is the contraction dim.
    s = np.sign(w).astype(NPBF16)
    t = s.reshape(MT, P, KT, P).transpose(0, 3, 2, 1)
    return np.ascontiguousarray(t).reshape(MT, P, KT * P)


def _prep_act(a, dtype):
    # [BL, F] -> feature-major [P, F//P, BL]: out[p, k, b] = a[b, k*P + p]
    t = a.T.reshape(-1, P, BL).transpose(1, 0, 2)
    return np.ascontiguousarray(t).astype(dtype)


def kernel(x, hidden, w_r, b_r, w_z, b_z, w_n, b_n):
    x = np.asarray(x, np.float32)
    hidden = np.asarray(hidden, np.float32)
    w_r, w_z, w_n = (np.asarray(w, np.float32) for w in (w_r, w_z, w_n))
    b_r, b_z, b_n = (np.asarray(b, np.float32) for b in (b_r, b_z, b_n))

    wr_t, wz_t, wn_t = _prep_weight(w_r), _prep_weight(w_z), _prep_weight(w_n)
    scales = np.array([np.mean(np.abs(w)) for w in (w_r, w_z, w_n)], np.float32)
    scale_arr = np.broadcast_to(scales, (P, 3)).copy()
    bias_arr = np.concatenate(
        [b.reshape(MT, P).T for b in (b_r, b_z, b_n)], axis=1
    ).astype(np.float32).copy()

    in_maps = []
    for c in range(NCORES):
        sl = slice(c * BL, (c + 1) * BL)
        in_maps.append({
            "xb": _prep_act(x[sl], NPBF16),
            "hb": _prep_act(hidden[sl], NPBF16),
            "hf": _prep_act(hidden[sl], np.float32),
            "wr": wr_t, "wz": wz_t, "wn": wn_t,
            "bias": bias_arr, "scale": scale_arr,
        })

    nc = _get_program()
    res = run_bass_kernel_spmd(nc, in_maps, core_ids=list(range(NCORES)))

    out = np.empty((B, H), np.float32)
    for c, r in enumerate(res.results):
        # [P, MT, BL] -> h_newT[m*P+p, b] -> [BL, H]
        o = r["out"].transpose(1, 0, 2).reshape(H, BL)
        out[c * BL:(c + 1) * BL] = o.T
    return out
